# revision 22
# baseline (speedup 1.0000x reference)
"""DeepSeekMoE (B=2, S=1024, H=1024, E=8, K=2, F=4096) on 8 trn2 cores.

Strategy (expert-parallel, host-routed):
  - Host computes the gate (sigmoid + top-2 + weight normalization) in
    float64 numpy -- it is ~0.01% of the FLOPs and determines the routing.
  - Tokens are gathered per expert on the host; core e runs expert e's FFN
    over its routed tokens (padded to capacity C, multiple of 128).
  - The shared expert is data-parallel: core e runs the shared FFN over
    tokens [e*256, (e+1)*256).
  - Device matmuls run in bf16 with fp32 PSUM accumulation; combine weights
    (and the 0.1 shared scale) are applied on-device during PSUM eviction.
  - Host scatters expert outputs back, adds the (token-weighted) expert
    down-bias and shared down-bias terms, and reshapes to [B, S, H].

Device layouts (all pre-packed on host so every DMA is 128-partition with
>=1KB contiguous per partition):
  xt   [H/128, 128, C]   xt[h, p, t]  = x_tok[t, 128h+p]          (bf16)
  wu   [F/128, 128, H]   wu[f, p, 128h+c] = Wu[128f+c, 128h+p]    (bf16)
  wd   [F/128, 128, H]   wd[f, p, n]  = Wd[n, 128f+p]             (bf16)
  bu   [128, F/128]      bu[p, f]     = bu_vec[128f+p]            (fp32)
  w    [128, C/128]      w[p, c]      = combine weight of token slot 128c+p
mm1: psum1[f] [128,Cc] += wu_sb[f][:,128h:128h+128].T @ xt_sb[h][:,Cc]
     a1[f] = gelu(psum1[f] + bu[:,f])                             (bf16)
mm2: psum2[c] [128,512] += a1[f][:,128c:128c+128].T @ wd_sb[f][:,512 half]
     y[c] = psum2[c] * w[:,c]  -> DRAM [C, H] fp32
"""

import numpy as np
import ml_dtypes

import concourse.bass as bass
import concourse.mybir as mybir
import concourse.tile as tile
from concourse import bacc
from concourse.bass_utils import run_bass_kernel_spmd

P = 128
B, S, H, E, K = 2, 1024, 1024, 8, 2
F = 4 * H
T = B * S
N_CORES = 8
CS = T // N_CORES  # shared-expert tokens per core

BF16 = mybir.dt.bfloat16
FP32 = mybir.dt.float32

_cache = {}
_pack_cache = {}


def _cached(key_arr, tag, fn):
    """Cache host-side weight packing keyed on the source array identity.
    The source array is pinned in the cache entry so its id stays valid."""
    k = (id(key_arr), tag)
    hit = _pack_cache.get(k)
    if hit is not None and hit[0] is key_arr:
        return hit[1]
    val = fn()
    _pack_cache[k] = (key_arr, val)
    return val


def _ffn_block(nc, pools, n_tok, h_dim, f_dim, xt_d, wu_d, wd_d, bu_d, w_d,
               out_d, scale_is_const, deferred_out, tags="", stream_wd=False,
               act=mybir.ActivationFunctionType.Gelu):
    """Emit one FFN (up -> gelu -> down, PSUM-evict with per-token scale).

    xt_d:[h_t,128,n_tok] wu_d:[f_t,128,h_dim] wd_d:[f_t,128,h_dim]
    bu_d:[128,f_t] w_d:[128,n_tok/128] (ignored when scale_is_const)
    out_d:[n_tok,h_dim]

    Pool tags are shared between successive calls so SBUF slots are reused
    across the expert and shared phases.
    """
    xt_pool, w_pool, a1_pool, wd_pool, y_pool, psum_pool, const_pool = pools
    h_t = h_dim // P
    f_t = f_dim // P
    c_t = n_tok // P
    # N-chunks of the token dim for mm1 (PSUM bank = 512 fp32)
    chunks = []
    off = 0
    while off < n_tok:
        n = min(512, n_tok - off)
        chunks.append((off, n))
        off += n

    # first up-proj weight tile ahead of the token tiles: the first matmul
    # needs xt[0] AND wu[0], and the single DMA queue is FIFO
    wu0_sb = w_pool.tile([P, h_dim], BF16, tag="wu", name="wu_first")
    nc.sync.dma_start(out=wu0_sb[:], in_=wu_d[0])

    # token-resident activations
    xt_sb = []
    for h in range(h_t):
        t_ = xt_pool.tile([P, n_tok], BF16, tag="xt", name=f"xt{h}")
        nc.sync.dma_start(out=t_[:], in_=xt_d[h])
        xt_sb.append(t_)

    bu_sb = const_pool.tile([P, f_t], FP32, tag="bu", name="bu_sb")
    nc.sync.dma_start(out=bu_sb[:], in_=bu_d[:])
    if not scale_is_const:
        w_sb = const_pool.tile([P, c_t], FP32, tag="w", name="w_sb")
        nc.sync.dma_start(out=w_sb[:], in_=w_d[:])

    # down-proj weights: resident when reused by >2 token tiles, else
    # streamed just-in-time through the wu pool during mm2 (f-outer).
    wd_sb = []
    if not stream_wd:
        for f in range(f_t):
            t_ = wd_pool.tile([P, h_dim], BF16, tag="wd", name=f"wd{f}")
            wd_sb.append(t_)

    # mm1 + gelu: a1[f] = gelu(Wu x + bu), kept resident for mm2
    a1_sb = []
    for f in range(f_t):
        if f == 0:
            wu_sb = wu0_sb
        else:
            wu_sb = w_pool.tile([P, h_dim], BF16, tag="wu", name=f"wu{f}")
            nc.sync.dma_start(out=wu_sb[:], in_=wu_d[f])
        if not stream_wd:
            nc.sync.dma_start(out=wd_sb[f][:], in_=wd_d[f])

        psums = [psum_pool.tile([P, n], FP32, tag=f"ps1_{ci}",
                                 name=f"ps1_{ci}_{f}")
                 for ci, (o, n) in enumerate(chunks)]
        for h in range(h_t):
            lhsT = wu_sb[:, h * P:(h + 1) * P]
            for ci, (o, n) in enumerate(chunks):
                nc.tensor.matmul(psums[ci][:], lhsT, xt_sb[h][:, o:o + n],
                                 start=(h == 0), stop=(h == h_t - 1))
        a1 = a1_pool.tile([P, n_tok], BF16, tag="a1", name=f"a1_{f}")
        for ci, (o, n) in enumerate(chunks):
            nc.scalar.activation(a1[:, o:o + n], psums[ci][:], act,
                                 bias=bu_sb[:, f:f + 1])
        a1_sb.append(a1)

    # mm2: out[c] = (sum_f a1[f][:,c].T @ wd[f]) * w[:,c]
    assert h_dim % 512 == 0
    n_half = h_dim // 512

    def evict(c, psums):
        y = y_pool.tile([P, h_dim], FP32, tag="y", name=f"y{tags}_{c}")
        scale = 0.1 if scale_is_const else w_sb[:, c:c + 1]
        for j in range(n_half):
            nc.scalar.activation(y[:, j * 512:(j + 1) * 512], psums[j][:],
                                 mybir.ActivationFunctionType.Copy,
                                 scale=scale)
        deferred_out.append((out_d[c * P:(c + 1) * P, :], y))

    if stream_wd:
        # f-outer / c-inner: needs c_t * n_half PSUM banks but streams wd
        # through the wu pool slots with no residency requirement
        assert c_t * n_half <= 4
        psums = [[psum_pool.tile([P, 512], FP32, tag=f"ps2_{j}",
                                  name=f"ps2_{j}_{c}")
                  for j in range(n_half)] for c in range(c_t)]
        for f in range(f_t):
            wd_f = wd_pool.tile([P, h_dim], BF16, tag="wd", name=f"wds{f}")
            nc.sync.dma_start(out=wd_f[:], in_=wd_d[f])
            for c in range(c_t):
                lhsT = a1_sb[f][:, c * P:(c + 1) * P]
                for j in range(n_half):
                    nc.tensor.matmul(psums[c][j][:], lhsT,
                                     wd_f[:, j * 512:(j + 1) * 512],
                                     start=(f == 0), stop=(f == f_t - 1))
        for c in range(c_t):
            evict(c, psums[c])
    else:
        for c in range(c_t):
            psums = [psum_pool.tile([P, 512], FP32, tag=f"ps2_{j}",
                                     name=f"ps2_{j}_{c}")
                     for j in range(n_half)]
            for f in range(f_t):
                lhsT = a1_sb[f][:, c * P:(c + 1) * P]
                for j in range(n_half):
                    nc.tensor.matmul(psums[j][:], lhsT,
                                     wd_sb[f][:, j * 512:(j + 1) * 512],
                                     start=(f == 0), stop=(f == f_t - 1))
            evict(c, psums)


def build_nc(C, h_dim=H, f_dim=F, cs=CS,
             act=mybir.ActivationFunctionType.Gelu):
    """Build the SPMD program: expert FFN over C routed tokens + shared FFN
    over cs tokens. All tensor names are fixed; per-core data differs."""
    nc = bacc.Bacc("TRN2", target_bir_lowering=False, debug=False,
                   enable_asserts=False)
    h_t, f_t = h_dim // P, f_dim // P

    d = {}
    d["xt"] = nc.dram_tensor("xt", [h_t, P, C], BF16, kind="ExternalInput").ap()
    d["wu"] = nc.dram_tensor("wu", [f_t, P, h_dim], BF16, kind="ExternalInput").ap()
    d["wd"] = nc.dram_tensor("wd", [f_t, P, h_dim], BF16, kind="ExternalInput").ap()
    d["bu"] = nc.dram_tensor("bu", [P, f_t], FP32, kind="ExternalInput").ap()
    d["w"] = nc.dram_tensor("w", [P, C // P], FP32, kind="ExternalInput").ap()
    d["xs"] = nc.dram_tensor("xs", [h_t, P, cs], BF16, kind="ExternalInput").ap()
    d["swu"] = nc.dram_tensor("swu", [f_t, P, h_dim], BF16, kind="ExternalInput").ap()
    d["swd"] = nc.dram_tensor("swd", [f_t, P, h_dim], BF16, kind="ExternalInput").ap()
    d["sbu"] = nc.dram_tensor("sbu", [P, f_t], FP32, kind="ExternalInput").ap()
    d["out_y"] = nc.dram_tensor("out_y", [C, h_dim], FP32, kind="ExternalOutput").ap()
    d["out_s"] = nc.dram_tensor("out_s", [cs, h_dim], FP32, kind="ExternalOutput").ap()

    with tile.TileContext(nc) as tc:
        with (
            tc.tile_pool(name="xt", bufs=h_t) as xt_pool,
            tc.tile_pool(name="wu", bufs=8) as w_pool,
            tc.tile_pool(name="a1", bufs=f_t) as a1_pool,
            tc.tile_pool(name="wd", bufs=f_t + 16) as wd_pool,
            tc.tile_pool(name="y", bufs=8) as y_pool,
            tc.tile_pool(name="psum", bufs=2, space="PSUM") as psum_pool,
            tc.tile_pool(name="const", bufs=2) as const_pool,
        ):
            pools = (xt_pool, w_pool, a1_pool, wd_pool, y_pool, psum_pool,
                     const_pool)
            deferred = []
            _ffn_block(nc, pools, C, h_dim, f_dim, d["xt"], d["wu"], d["wd"],
                       d["bu"], d["w"], d["out_y"], False, deferred,
                       tags="e", act=act)
            _ffn_block(nc, pools, cs, h_dim, f_dim, d["xs"], d["swu"],
                       d["swd"], d["sbu"], None, d["out_s"], True, deferred,
                       tags="s", stream_wd=True, act=act)
            for dst, y in deferred:
                nc.sync.dma_start(out=dst, in_=y[:])
    nc.compile()
    return nc


def _route(x_flat, gate_w, gate_bias):
    """Host gate: float64 sigmoid + top-2 + normalized combine weights."""
    logits = x_flat.astype(np.float64) @ gate_w.astype(np.float64).T
    logits += gate_bias.astype(np.float64)
    scores = 1.0 / (1.0 + np.exp(-logits))
    top_idx = np.argsort(-scores, axis=1, kind="stable")[:, :K]
    rows = np.arange(x_flat.shape[0])[:, None]
    top_sc = scores[rows, top_idx]
    wts = top_sc / (top_sc.sum(-1, keepdims=True) + 1e-6)
    return top_idx, wts.astype(np.float32)


def _pack_wu(Wu_e):
    f_t, h_t = Wu_e.shape[0] // P, Wu_e.shape[1] // P
    return np.ascontiguousarray(
        Wu_e.reshape(f_t, P, h_t, P).transpose(0, 3, 2, 1)
        .reshape(f_t, P, h_t * P).astype(ml_dtypes.bfloat16))


def _pack_wd(Wd_e):
    f_t = Wd_e.shape[1] // P
    return np.ascontiguousarray(
        Wd_e.T.reshape(f_t, P, Wd_e.shape[0]).astype(ml_dtypes.bfloat16))


def _pack_x(x_tok, C):
    n, h_dim = x_tok.shape
    h_t = h_dim // P
    xp = np.zeros((C, h_dim), np.float32)
    xp[:n] = x_tok
    return np.ascontiguousarray(
        xp.reshape(C, h_t, P).transpose(1, 2, 0).astype(ml_dtypes.bfloat16))


def _pack_bias(b):
    return np.ascontiguousarray(b.reshape(-1, P).T.astype(np.float32))


def kernel(x, gate_w, gate_bias, Wu, bu, Wd, bd, sWu, sbu, sWd, sbd):
    x = np.asarray(x, np.float32)
    x_flat = x.reshape(-1, x.shape[-1])

    top_idx, wts = _route(x_flat, np.asarray(gate_w), np.asarray(gate_bias))

    # per-expert token lists
    idx_e, w_e = [], []
    for e in range(E):
        sel = np.nonzero((top_idx == e).any(axis=1))[0]
        slot = (top_idx[sel] == e).argmax(axis=1)
        idx_e.append(sel)
        w_e.append(wts[sel, slot])
    counts = np.array([len(i) for i in idx_e])
    C = max(P, int(-(-counts.max() // P)) * P)

    if C not in _cache:
        _cache[C] = build_nc(C)
    nc = _cache[C]

    swu_p = _cached(sWu, "swu", lambda: _pack_wu(np.asarray(sWu, np.float32)))
    swd_p = _cached(sWd, "swd", lambda: _pack_wd(np.asarray(sWd, np.float32)))
    sbu_p = _pack_bias(np.asarray(sbu, np.float32))
    wu_packs = _cached(Wu, "wu", lambda: [
        _pack_wu(np.asarray(Wu, np.float32)[e]) for e in range(E)])
    wd_packs = _cached(Wd, "wd", lambda: [
        _pack_wd(np.asarray(Wd, np.float32)[e]) for e in range(E)])

    in_maps = []
    for e in range(E):
        wv = np.zeros(C, np.float32)
        wv[:counts[e]] = w_e[e]
        in_maps.append({
            "xt": _pack_x(x_flat[idx_e[e]], C),
            "wu": wu_packs[e],
            "wd": wd_packs[e],
            "bu": _pack_bias(np.asarray(bu, np.float32)[e]),
            "w": np.ascontiguousarray(wv.reshape(-1, P).T),
            "xs": _pack_x(x_flat[e * CS:(e + 1) * CS], CS),
            "swu": swu_p,
            "swd": swd_p,
            "sbu": sbu_p,
        })

    res = run_bass_kernel_spmd(nc, in_maps, core_ids=list(range(N_CORES)))
    kernel.last_results = res

    out = np.zeros((T, H), np.float32)
    for e in range(E):
        out[idx_e[e]] += res.results[e]["out_y"][:counts[e]]
        out[e * CS:(e + 1) * CS] += res.results[e]["out_s"]
    # expert down-bias: sum_k w[t,k] * bd[e_k]; shared down-bias * 0.1
    w_dense = np.zeros((T, E), np.float32)
    w_dense[np.arange(T)[:, None], top_idx] = wts
    out += w_dense @ np.asarray(bd, np.float32)
    out += 0.1 * np.asarray(sbd, np.float32)[None, :]
    return out.reshape(B, S, H)


# revision 23
# speedup vs baseline: 1.0153x; 1.0153x over previous
"""DeepSeekMoE (B=2, S=1024, H=1024, E=8, K=2, F=4096) on 8 trn2 cores.

Strategy (expert-parallel, host-routed):
  - Host computes the gate (sigmoid + top-2 + weight normalization) in
    float64 numpy -- it is ~0.01% of the FLOPs and determines the routing.
  - Tokens are gathered per expert on the host; core e runs expert e's FFN
    over its routed tokens (padded to capacity C, multiple of 128).
  - The shared expert is data-parallel: core e runs the shared FFN over
    tokens [e*256, (e+1)*256).
  - Device matmuls run in bf16 with fp32 PSUM accumulation; combine weights
    (and the 0.1 shared scale) are applied on-device during PSUM eviction.
  - Host scatters expert outputs back, adds the (token-weighted) expert
    down-bias and shared down-bias terms, and reshapes to [B, S, H].

Device layouts (all pre-packed on host so every DMA is 128-partition with
>=1KB contiguous per partition):
  xt   [H/128, 128, C]   xt[h, p, t]  = x_tok[t, 128h+p]          (bf16)
  wu   [F/128, 128, H]   wu[f, p, 128h+c] = Wu[128f+c, 128h+p]    (bf16)
  wd   [F/128, 128, H]   wd[f, p, n]  = Wd[n, 128f+p]             (bf16)
  bu   [128, F/128]      bu[p, f]     = bu_vec[128f+p]            (fp32)
  w    [128, C/128]      w[p, c]      = combine weight of token slot 128c+p
mm1: psum1[f] [128,Cc] += wu_sb[f][:,128h:128h+128].T @ xt_sb[h][:,Cc]
     a1[f] = gelu(psum1[f] + bu[:,f])                             (bf16)
mm2: psum2[c] [128,512] += a1[f][:,128c:128c+128].T @ wd_sb[f][:,512 half]
     y[c] = psum2[c] * w[:,c]  -> DRAM [C, H] fp32
"""

import numpy as np
import ml_dtypes

import concourse.bass as bass
import concourse.mybir as mybir
import concourse.tile as tile
from concourse import bacc
from concourse.bass_utils import run_bass_kernel_spmd

P = 128
B, S, H, E, K = 2, 1024, 1024, 8, 2
F = 4 * H
T = B * S
N_CORES = 8
CS = T // N_CORES  # shared-expert tokens per core

BF16 = mybir.dt.bfloat16
FP32 = mybir.dt.float32

_cache = {}
_pack_cache = {}


def _cached(key_arr, tag, fn):
    """Cache host-side weight packing keyed on the source array identity.
    The source array is pinned in the cache entry so its id stays valid."""
    k = (id(key_arr), tag)
    hit = _pack_cache.get(k)
    if hit is not None and hit[0] is key_arr:
        return hit[1]
    val = fn()
    _pack_cache[k] = (key_arr, val)
    return val


def _ffn_block(nc, pools, n_tok, h_dim, f_dim, xt_d, wu_d, wd_d, bu_d, w_d,
               out_d, scale_is_const, deferred_out, tags="", stream_wd=False,
               act=mybir.ActivationFunctionType.Gelu):
    """Emit one FFN (up -> gelu -> down, PSUM-evict with per-token scale).

    xt_d:[h_t,128,n_tok] wu_d:[f_t,128,h_dim] wd_d:[f_t,128,h_dim]
    bu_d:[128,f_t] w_d:[128,n_tok/128] (ignored when scale_is_const)
    out_d:[n_tok,h_dim]

    Pool tags are shared between successive calls so SBUF slots are reused
    across the expert and shared phases.
    """
    xt_pool, w_pool, a1_pool, wd_pool, y_pool, psum_pool, const_pool = pools
    h_t = h_dim // P
    f_t = f_dim // P
    c_t = n_tok // P
    # N-chunks of the token dim for mm1 (PSUM bank = 512 fp32)
    chunks = []
    off = 0
    while off < n_tok:
        n = min(512, n_tok - off)
        chunks.append((off, n))
        off += n

    # first up-proj weight tile ahead of the token tiles: the first matmul
    # needs xt[0] AND wu[0], and the single DMA queue is FIFO
    wu0_sb = w_pool.tile([P, h_dim], BF16, tag="wu", name="wu_first")
    nc.sync.dma_start(out=wu0_sb[:], in_=wu_d[0])

    # token-resident activations
    xt_sb = []
    for h in range(h_t):
        t_ = xt_pool.tile([P, n_tok], BF16, tag="xt", name=f"xt{h}")
        nc.sync.dma_start(out=t_[:], in_=xt_d[h])
        xt_sb.append(t_)

    bu_sb = const_pool.tile([P, f_t], FP32, tag="bu", name="bu_sb")
    nc.sync.dma_start(out=bu_sb[:], in_=bu_d[:])
    if not scale_is_const:
        w_sb = const_pool.tile([P, c_t], FP32, tag="w", name="w_sb")
        nc.sync.dma_start(out=w_sb[:], in_=w_d[:])

    # down-proj weights: resident when reused by >2 token tiles, else
    # streamed just-in-time through the wu pool during mm2 (f-outer).
    wd_sb = []
    if not stream_wd:
        for f in range(f_t):
            t_ = wd_pool.tile([P, h_dim], BF16, tag="wd", name=f"wd{f}")
            wd_sb.append(t_)

    # mm1 + gelu: a1[f] = gelu(Wu x + bu), kept resident for mm2
    a1_sb = []
    for f in range(f_t):
        if f == 0:
            wu_sb = wu0_sb
        else:
            wu_sb = w_pool.tile([P, h_dim], BF16, tag="wu", name=f"wu{f}")
            nc.sync.dma_start(out=wu_sb[:], in_=wu_d[f])
        if not stream_wd:
            nc.sync.dma_start(out=wd_sb[f][:], in_=wd_d[f])

        psums = [psum_pool.tile([P, n], FP32, tag=f"ps1_{ci}",
                                 name=f"ps1_{ci}_{f}")
                 for ci, (o, n) in enumerate(chunks)]
        for h in range(h_t):
            lhsT = wu_sb[:, h * P:(h + 1) * P]
            for ci, (o, n) in enumerate(chunks):
                nc.tensor.matmul(psums[ci][:], lhsT, xt_sb[h][:, o:o + n],
                                 start=(h == 0), stop=(h == h_t - 1))
        a1 = a1_pool.tile([P, n_tok], BF16, tag="a1", name=f"a1_{f}")
        for ci, (o, n) in enumerate(chunks):
            nc.scalar.activation(a1[:, o:o + n], psums[ci][:], act,
                                 bias=bu_sb[:, f:f + 1])
        a1_sb.append(a1)

    # mm2: out[c] = (sum_f a1[f][:,c].T @ wd[f]) * w[:,c]
    assert h_dim % 512 == 0
    n_half = h_dim // 512

    def evict(c, psums):
        y = y_pool.tile([P, h_dim], FP32, tag="y", name=f"y{tags}_{c}")
        scale = 0.1 if scale_is_const else w_sb[:, c:c + 1]
        for j in range(n_half):
            nc.scalar.activation(y[:, j * 512:(j + 1) * 512], psums[j][:],
                                 mybir.ActivationFunctionType.Copy,
                                 scale=scale)
        deferred_out.append((out_d[c * P:(c + 1) * P, :], y))

    if stream_wd:
        # f-outer / c-inner: needs c_t * n_half PSUM banks but streams wd
        # through the wu pool slots with no residency requirement
        assert c_t * n_half <= 4
        psums = [[psum_pool.tile([P, 512], FP32, tag=f"ps2_{j}",
                                  name=f"ps2_{j}_{c}")
                  for j in range(n_half)] for c in range(c_t)]
        for f in range(f_t):
            wd_f = wd_pool.tile([P, h_dim], BF16, tag="wd", name=f"wds{f}")
            nc.sync.dma_start(out=wd_f[:], in_=wd_d[f])
            for c in range(c_t):
                lhsT = a1_sb[f][:, c * P:(c + 1) * P]
                for j in range(n_half):
                    nc.tensor.matmul(psums[c][j][:], lhsT,
                                     wd_f[:, j * 512:(j + 1) * 512],
                                     start=(f == 0), stop=(f == f_t - 1))
        # flush earlier-phase output DMAs here: their data is ready by now,
        # so they slot into the sync stream without head-of-line waits
        for dst, y in list(deferred_out):
            nc.sync.dma_start(out=dst, in_=y[:])
        deferred_out.clear()
        for c in range(c_t):
            evict(c, psums[c])
    else:
        for c in range(c_t):
            psums = [psum_pool.tile([P, 512], FP32, tag=f"ps2_{j}",
                                     name=f"ps2_{j}_{c}")
                     for j in range(n_half)]
            for f in range(f_t):
                lhsT = a1_sb[f][:, c * P:(c + 1) * P]
                for j in range(n_half):
                    nc.tensor.matmul(psums[j][:], lhsT,
                                     wd_sb[f][:, j * 512:(j + 1) * 512],
                                     start=(f == 0), stop=(f == f_t - 1))
            evict(c, psums)


def build_nc(C, h_dim=H, f_dim=F, cs=CS,
             act=mybir.ActivationFunctionType.Gelu):
    """Build the SPMD program: expert FFN over C routed tokens + shared FFN
    over cs tokens. All tensor names are fixed; per-core data differs."""
    nc = bacc.Bacc("TRN2", target_bir_lowering=False, debug=False,
                   enable_asserts=False)
    h_t, f_t = h_dim // P, f_dim // P

    d = {}
    d["xt"] = nc.dram_tensor("xt", [h_t, P, C], BF16, kind="ExternalInput").ap()
    d["wu"] = nc.dram_tensor("wu", [f_t, P, h_dim], BF16, kind="ExternalInput").ap()
    d["wd"] = nc.dram_tensor("wd", [f_t, P, h_dim], BF16, kind="ExternalInput").ap()
    d["bu"] = nc.dram_tensor("bu", [P, f_t], FP32, kind="ExternalInput").ap()
    d["w"] = nc.dram_tensor("w", [P, C // P], FP32, kind="ExternalInput").ap()
    d["xs"] = nc.dram_tensor("xs", [h_t, P, cs], BF16, kind="ExternalInput").ap()
    d["swu"] = nc.dram_tensor("swu", [f_t, P, h_dim], BF16, kind="ExternalInput").ap()
    d["swd"] = nc.dram_tensor("swd", [f_t, P, h_dim], BF16, kind="ExternalInput").ap()
    d["sbu"] = nc.dram_tensor("sbu", [P, f_t], FP32, kind="ExternalInput").ap()
    d["out_y"] = nc.dram_tensor("out_y", [C, h_dim], FP32, kind="ExternalOutput").ap()
    d["out_s"] = nc.dram_tensor("out_s", [cs, h_dim], FP32, kind="ExternalOutput").ap()

    with tile.TileContext(nc) as tc:
        with (
            tc.tile_pool(name="xt", bufs=h_t) as xt_pool,
            tc.tile_pool(name="wu", bufs=8) as w_pool,
            tc.tile_pool(name="a1", bufs=f_t) as a1_pool,
            tc.tile_pool(name="wd", bufs=f_t + 16) as wd_pool,
            tc.tile_pool(name="y", bufs=8) as y_pool,
            tc.tile_pool(name="psum", bufs=2, space="PSUM") as psum_pool,
            tc.tile_pool(name="const", bufs=2) as const_pool,
        ):
            pools = (xt_pool, w_pool, a1_pool, wd_pool, y_pool, psum_pool,
                     const_pool)
            deferred = []
            _ffn_block(nc, pools, C, h_dim, f_dim, d["xt"], d["wu"], d["wd"],
                       d["bu"], d["w"], d["out_y"], False, deferred,
                       tags="e", act=act)
            _ffn_block(nc, pools, cs, h_dim, f_dim, d["xs"], d["swu"],
                       d["swd"], d["sbu"], None, d["out_s"], True, deferred,
                       tags="s", stream_wd=True, act=act)
            for dst, y in deferred:
                nc.sync.dma_start(out=dst, in_=y[:])
    nc.compile()
    return nc


def _route(x_flat, gate_w, gate_bias):
    """Host gate: float64 sigmoid + top-2 + normalized combine weights."""
    logits = x_flat.astype(np.float64) @ gate_w.astype(np.float64).T
    logits += gate_bias.astype(np.float64)
    scores = 1.0 / (1.0 + np.exp(-logits))
    top_idx = np.argsort(-scores, axis=1, kind="stable")[:, :K]
    rows = np.arange(x_flat.shape[0])[:, None]
    top_sc = scores[rows, top_idx]
    wts = top_sc / (top_sc.sum(-1, keepdims=True) + 1e-6)
    return top_idx, wts.astype(np.float32)


def _pack_wu(Wu_e):
    f_t, h_t = Wu_e.shape[0] // P, Wu_e.shape[1] // P
    return np.ascontiguousarray(
        Wu_e.reshape(f_t, P, h_t, P).transpose(0, 3, 2, 1)
        .reshape(f_t, P, h_t * P).astype(ml_dtypes.bfloat16))


def _pack_wd(Wd_e):
    f_t = Wd_e.shape[1] // P
    return np.ascontiguousarray(
        Wd_e.T.reshape(f_t, P, Wd_e.shape[0]).astype(ml_dtypes.bfloat16))


def _pack_x(x_tok, C):
    n, h_dim = x_tok.shape
    h_t = h_dim // P
    xp = np.zeros((C, h_dim), np.float32)
    xp[:n] = x_tok
    return np.ascontiguousarray(
        xp.reshape(C, h_t, P).transpose(1, 2, 0).astype(ml_dtypes.bfloat16))


def _pack_bias(b):
    return np.ascontiguousarray(b.reshape(-1, P).T.astype(np.float32))


def kernel(x, gate_w, gate_bias, Wu, bu, Wd, bd, sWu, sbu, sWd, sbd):
    x = np.asarray(x, np.float32)
    x_flat = x.reshape(-1, x.shape[-1])

    top_idx, wts = _route(x_flat, np.asarray(gate_w), np.asarray(gate_bias))

    # per-expert token lists
    idx_e, w_e = [], []
    for e in range(E):
        sel = np.nonzero((top_idx == e).any(axis=1))[0]
        slot = (top_idx[sel] == e).argmax(axis=1)
        idx_e.append(sel)
        w_e.append(wts[sel, slot])
    counts = np.array([len(i) for i in idx_e])
    C = max(P, int(-(-counts.max() // P)) * P)

    if C not in _cache:
        _cache[C] = build_nc(C)
    nc = _cache[C]

    swu_p = _cached(sWu, "swu", lambda: _pack_wu(np.asarray(sWu, np.float32)))
    swd_p = _cached(sWd, "swd", lambda: _pack_wd(np.asarray(sWd, np.float32)))
    sbu_p = _pack_bias(np.asarray(sbu, np.float32))
    wu_packs = _cached(Wu, "wu", lambda: [
        _pack_wu(np.asarray(Wu, np.float32)[e]) for e in range(E)])
    wd_packs = _cached(Wd, "wd", lambda: [
        _pack_wd(np.asarray(Wd, np.float32)[e]) for e in range(E)])

    in_maps = []
    for e in range(E):
        wv = np.zeros(C, np.float32)
        wv[:counts[e]] = w_e[e]
        in_maps.append({
            "xt": _pack_x(x_flat[idx_e[e]], C),
            "wu": wu_packs[e],
            "wd": wd_packs[e],
            "bu": _pack_bias(np.asarray(bu, np.float32)[e]),
            "w": np.ascontiguousarray(wv.reshape(-1, P).T),
            "xs": _pack_x(x_flat[e * CS:(e + 1) * CS], CS),
            "swu": swu_p,
            "swd": swd_p,
            "sbu": sbu_p,
        })

    res = run_bass_kernel_spmd(nc, in_maps, core_ids=list(range(N_CORES)))
    kernel.last_results = res

    out = np.zeros((T, H), np.float32)
    for e in range(E):
        out[idx_e[e]] += res.results[e]["out_y"][:counts[e]]
        out[e * CS:(e + 1) * CS] += res.results[e]["out_s"]
    # expert down-bias: sum_k w[t,k] * bd[e_k]; shared down-bias * 0.1
    w_dense = np.zeros((T, E), np.float32)
    w_dense[np.arange(T)[:, None], top_idx] = wts
    out += w_dense @ np.asarray(bd, np.float32)
    out += 0.1 * np.asarray(sbd, np.float32)[None, :]
    return out.reshape(B, S, H)
